# revision 41
# baseline (speedup 1.0000x reference)
"""Single-head causal attention kernel for Trainium2 (Bass/Tile), SPMD over 8 cores.

Problem: inputs [B=8, S=2048, E=1024]; Wq/Wk/Wv [E, H=1024]; bq/bk/bv [H].
  q = x@Wq+bq; k = x@Wk+bk; v = x@Wv+bv
  out = softmax(causal(q k^T / sqrt(H))) v        -> [B, S, H]

Sharding: data-parallel over batch, 1 batch element per NeuronCore (8 cores).

Strategy (v3, bf16): host passes x pre-transposed (xT [E,S]) and weights in
bf16 (Wq/Wk additionally pre-tiled by output h-tile so the first matmul only
depends on a 256KB DMA), so the device does zero transposes and keeps
everything resident in SBUF:
  phase 1 (per 512-wide s-chunk): K^T[h,s] and Q^T[h,s] (stationary W tiles,
           bias fused into the PSUM eviction), then V[s,h] (stationary xT
           tiles; bias added during eviction from a host-broadcast [128,H]
           bv tile on GpSimd). All matmuls N=512.
  phase 2 (q-chunks processed in reverse so the tail chain is the smallest):
           scoresT[k,q] matmuls, exp(x/32) on ScalarE, causal edge mask via
           gpsimd.affine_select; AV + Z share one stationary load per attnT
           tile (fully-masked diagonal tiles skipped for even q-subtiles);
           1/Z folded into the PSUM eviction. AV of a chunk is issued after
           the next chunk's scores so the PE in-order queue never stalls on
           ScalarE.
  A short burst of dummy matmuls runs during the initial DMA wait to lift
  the PE HAM clock-gate to 8/8 before real work starts.
"""

import numpy as np
import ml_dtypes

import concourse.bacc as bacc
import concourse.mybir as mybir
from concourse import tile
from concourse import bass_utils

P = 128
F32 = mybir.dt.float32
BF16 = mybir.dt.bfloat16

B, S, E, H = 8, 2048, 1024, 1024
QC = 256          # q-chunk width in attention phase
N_CORES = 8
NPBF16 = ml_dtypes.bfloat16


def attention_kernel(tc, out, xt, wqp, bq, wkp, bk, wv, bvb):
    nc = tc.nc
    ST, ET, HT = S // P, E // P, H // P     # 128-tiles per dim
    NSC = S // 512                          # 512-wide s-chunks
    NQC = S // QC                           # q-chunks
    QSUB = QC // P
    inv_sqrt_h = 1.0 / float(np.sqrt(H))

    from contextlib import ExitStack

    root = ExitStack()
    with root:
        # ---- constants ----
        const = root.enter_context(tc.tile_pool(name="const", bufs=1))
        ones_col = const.tile([P, 1], BF16, name="ones_col")
        nc.gpsimd.memset(ones_col, 1.0)
        warm_src = const.tile([P, 512], BF16, name="warm_src")
        nc.gpsimd.memset(warm_src, 0.0)
        bk_sb = const.tile([P, HT], F32, name="bk_sb")
        nc.sync.dma_start(bk_sb[:], bk.rearrange("(t p) -> p t", p=P))
        bq_sb = const.tile([P, HT], F32, name="bq_sb")
        nc.sync.dma_start(bq_sb[:], bq.rearrange("(t p) -> p t", p=P))
        bv_sb = const.tile([P, H], BF16, name="bv_sb")

        # ---- resident arrays ----
        kqv_pool = root.enter_context(tc.tile_pool(name="kqv", bufs=1))
        kt = kqv_pool.tile([P, HT, S], BF16, name="kt")     # K^T [h,s]
        qt = kqv_pool.tile([P, HT, S], BF16, name="qt")     # Q^T [h,s]
        v_sb = kqv_pool.tile([P, ST, H], BF16, name="v_sb")  # V [s,h]

        # ================= phase 1: projections =================
        with ExitStack() as ph1:
            w_pool = ph1.enter_context(tc.tile_pool(name="w", bufs=1))
            # wk_sb[:, t, e, :] = Wk[e*128+p, t*128+c]  (host pre-tiled)
            wk_sb = w_pool.tile([P, HT, ET, P], BF16, name="wk_sb")
            wq_sb = w_pool.tile([P, HT, ET, P], BF16, name="wq_sb")
            wv_sb = w_pool.tile([P, ET, H], BF16, name="wv_sb")
            xt_pool = ph1.enter_context(tc.tile_pool(name="xt", bufs=1))
            # chunk-major: [:, c, e, :] is one contiguous 8KB/partition DMA
            xt_sb = xt_pool.tile([P, NSC, ET, 512], BF16, name="xt_sb")

            # DMA priority order (single queue => sequential arrival):
            # xt chunk0 + wk (first K matmuls), then wv, wq, xt chunks 1-3.
            CW = ET * 512                        # xtp columns per chunk
            # chunk 0 split per e-tile for fine-grained matmul wakeup
            nc.sync.dma_start(xt_sb[:, 0, 0, :], xt[:, 0:512])
            nc.sync.dma_start(wk_sb[:, 0, :, :], wkp[0:P, :])
            for e in range(1, ET):
                nc.sync.dma_start(xt_sb[:, 0, e, :],
                                  xt[:, e * 512:(e + 1) * 512])
            for t in range(1, HT):
                nc.sync.dma_start(wk_sb[:, t, :, :], wkp[t * P:(t + 1) * P, :])
            for e in range(ET):
                nc.sync.dma_start(wv_sb[:, e, :], wv[e * P:(e + 1) * P, :])
            nc.sync.dma_start(bv_sb[:], bvb)
            for t in range(HT):
                nc.sync.dma_start(wq_sb[:, t, :, :], wqp[t * P:(t + 1) * P, :])
            for c in range(1, NSC):
                nc.sync.dma_start(xt_sb[:, c, :, :],
                                  xt[:, c * CW:(c + 1) * CW])

            # kqpsum first: phase-2 pools then reuse banks that phase 1
            # frees early (after Q) rather than last (after V's evictions).
            kqpsum = ph1.enter_context(tc.tile_pool(name="kqpsum", bufs=2,
                                                    space="PSUM"))
            vpsum = ph1.enter_context(tc.tile_pool(name="vpsum", bufs=2,
                                                   space="PSUM"))
            # HAM warmup: dummy matmuls with no DMA dependency fill the
            # initial DMA wait and lift the PE clock gate to 8/8.
            wp = kqpsum.tile([P, 512], F32, name="kq0", space="PSUM")
            for _ in range(22):
                nc.tensor.matmul(wp[:], warm_src[:, 0:P], warm_src[:],
                                 start=True, stop=True)

            def evict_kq(dst, t, c, psum, bias, alt):
                if alt % 2 == 0:
                    nc.scalar.activation(
                        dst[:, t, c * 512:(c + 1) * 512], psum[:],
                        mybir.ActivationFunctionType.Identity,
                        bias=bias[:, t:t + 1])
                else:
                    nc.vector.tensor_scalar_add(
                        dst[:, t, c * 512:(c + 1) * 512], psum[:],
                        bias[:, t:t + 1])

            def v_chunk(c):
                for si in range(4 * c, 4 * c + 4):
                    vps = []
                    for hc in range(2):
                        vp = vpsum.tile([P, 512], F32, name="vp", space="PSUM")
                        for e in range(ET):
                            nc.tensor.matmul(
                                vp[:],
                                xt_sb[:, c, e, (si % 4) * P:(si % 4 + 1) * P],
                                wv_sb[:, e, hc * 512:(hc + 1) * 512],
                                start=(e == 0), stop=(e == ET - 1))
                        vps.append(vp)
                    for hc in range(2):
                        nc.vector.scalar_tensor_tensor(
                            v_sb[:, si, hc * 512:(hc + 1) * 512], vps[hc][:],
                            1.0, bv_sb[:, hc * 512:(hc + 1) * 512],
                            mybir.AluOpType.mult, mybir.AluOpType.add)

            # ---- chunk 0: chunk-local (DMA-paced at startup) ----
            for t in range(HT):
                kp = kqpsum.tile([P, 512], F32, name="kq0", space="PSUM")
                for e in range(ET):
                    nc.tensor.matmul(
                        kp[:], wk_sb[:, t, e, :], xt_sb[:, 0, e, :],
                        start=(e == 0), stop=(e == ET - 1))
                evict_kq(kt, t, 0, kp, bk_sb, t)
            v_chunk(0)
            for t in range(HT):
                qp = kqpsum.tile([P, 512], F32, name="kq1", space="PSUM")
                for e in range(ET):
                    nc.tensor.matmul(
                        qp[:], wq_sb[:, t, e, :], xt_sb[:, 0, e, :],
                        start=(e == 0), stop=(e == ET - 1))
                evict_kq(qt, t, 0, qp, bq_sb, t)
            # ---- chunks 1-3: reuse each stationary W tile across chunks ----
            for dst, w_sb, b_sb in ((kt, wk_sb, bk_sb), (qt, wq_sb, bq_sb)):
                for t in range(HT):
                    kps = [kqpsum.tile([P, 512], F32, name=f"kq{cc}",
                                       space="PSUM") for cc in range(3)]
                    for e in range(ET):
                        for cc in range(3):
                            nc.tensor.matmul(
                                kps[cc][:], w_sb[:, t, e, :],
                                xt_sb[:, 1 + cc, e, :],
                                start=(e == 0), stop=(e == ET - 1))
                    for cc in range(3):
                        evict_kq(dst, t, 1 + cc, kps[cc], b_sb, t + cc)
            for c in range(1, NSC):
                v_chunk(c)

        # ================= phase 2: attention =================
        with ExitStack() as ph2:
            attn_pool = ph2.enter_context(
                tc.tile_pool(name="attnT", bufs=36))
            o_pool = ph2.enter_context(tc.tile_pool(name="o_stage", bufs=3))
            rz_pool = ph2.enter_context(tc.tile_pool(name="rz", bufs=4))
            spsum = ph2.enter_context(tc.tile_pool(name="spsum", bufs=2,
                                                   space="PSUM"))
            zpsum = ph2.enter_context(tc.tile_pool(name="zpsum", bufs=2,
                                                   space="PSUM"))
            opsum = ph2.enter_context(tc.tile_pool(name="opsum", bufs=4,
                                                   space="PSUM"))

            def scores_chunk(j):
                """ScoresT tiles [k,q] + exp + causal mask for q-chunk j."""
                nk = ((j + 1) * QC) // P
                ats = []
                for i in range(nk):
                    # The last k-tile (i == 2j+1) lies above the diagonal for
                    # the first q-subtile; only its [:, P:2P] half is ever
                    # read by AV/Z, so compute just those 128 columns.
                    lo = P if i == nk - 1 else 0
                    sp = spsum.tile([P, QC], F32, name="sp", space="PSUM")
                    for t in range(HT):
                        nc.tensor.matmul(
                            sp[:, 0:QC - lo],
                            kt[:, t, i * P:(i + 1) * P],
                            qt[:, t, j * QC + lo:(j + 1) * QC],
                            start=(t == 0), stop=(t == HT - 1))
                    at = attn_pool.tile([P, QC], BF16, name="at")
                    nc.scalar.activation(at[:, lo:QC], sp[:, 0:QC - lo],
                                         mybir.ActivationFunctionType.Exp,
                                         scale=inv_sqrt_h)
                    if (i + 1) * P > j * QC + lo:   # tile touches the diagonal
                        nc.gpsimd.affine_select(
                            out=at[:, lo:QC], in_=at[:, lo:QC],
                            compare_op=mybir.AluOpType.is_ge,
                            fill=0.0,
                            base=j * QC + lo - i * P,
                            channel_multiplier=-1,
                            pattern=[[1, QC - lo]])
                    ats.append(at)
                return ats

            def av_chunk(j, ats):
                """AV + Z for q-chunk j given its masked attnT tiles."""
                for qs in range(QSUB):
                    # causal: k-tiles above the diagonal for this q-subtile
                    # are fully masked; skip them.
                    nk = 2 * j + qs + 1
                    zp = zpsum.tile([P, 1], F32, name="zp", space="PSUM")
                    ops = [opsum.tile([P, 512], F32, name="op", space="PSUM")
                           for _ in range(2)]
                    for i in range(nk):
                        a_sl = ats[i][:, qs * P:(qs + 1) * P]
                        for hc in range(2):
                            nc.tensor.matmul(
                                ops[hc][:], a_sl,
                                v_sb[:, i, hc * 512:(hc + 1) * 512],
                                start=(i == 0), stop=(i == nk - 1))
                        nc.tensor.matmul(zp[:], a_sl, ones_col[:, :],
                                         start=(i == 0), stop=(i == nk - 1))
                    rz = rz_pool.tile([P, 1], F32, name="rz")
                    nc.vector.reciprocal(rz[:], zp[:])
                    o_st = o_pool.tile([P, H], BF16, name="o_st")
                    row = j * QC + qs * P
                    for hc in range(2):
                        nc.vector.tensor_scalar_mul(
                            o_st[:, hc * 512:(hc + 1) * 512], ops[hc][:],
                            rz[:, 0:1])
                        nc.sync.dma_start(
                            out[row:row + P, hc * 512:(hc + 1) * 512],
                            o_st[:, hc * 512:(hc + 1) * 512])

            prev = None
            prev_j = None
            for j in range(NQC - 1, -1, -1):     # reverse: smallest AV last
                ats = scores_chunk(j)
                if prev is not None:
                    av_chunk(prev_j, prev)
                prev, prev_j = ats, j
            av_chunk(prev_j, prev)


def build_program(n_cores=N_CORES):
    nc = bacc.Bacc("TRN2", target_bir_lowering=False, debug=False,
                   num_devices=n_cores)
    xt = nc.dram_tensor("xt", [P, S * E // P], BF16, kind="ExternalInput").ap()
    wqp = nc.dram_tensor("wqp", [H, E], BF16, kind="ExternalInput").ap()
    bq = nc.dram_tensor("bq", [H], F32, kind="ExternalInput").ap()
    wkp = nc.dram_tensor("wkp", [H, E], BF16, kind="ExternalInput").ap()
    bk = nc.dram_tensor("bk", [H], F32, kind="ExternalInput").ap()
    wv = nc.dram_tensor("wv", [E, H], BF16, kind="ExternalInput").ap()
    bvb = nc.dram_tensor("bvb", [P, H], BF16, kind="ExternalInput").ap()
    out = nc.dram_tensor("out", [S, H], BF16, kind="ExternalOutput").ap()
    with tile.TileContext(nc) as tc:
        attention_kernel(tc, out, xt, wqp, bq, wkp, bk, wv, bvb)
    nc.compile()
    return nc


def _tile_by_h(w):
    """[E,H] -> [H,E] layout where row t*128+p, col e*128+c = w[e*128+p, t*128+c].

    So a [128, E] slice at row offset t*128 holds, for partition p, the
    concatenation over e of Wk[e*128+p, t*128:(t+1)*128].
    """
    w4 = w.reshape(E // P, P, H // P, P)          # [e, p, t, c]
    return np.ascontiguousarray(
        w4.transpose(2, 1, 0, 3).reshape(H, E))   # [t, p, e, c] -> [H, E]


def _pack_xt(x):
    """x [S,E] -> xtp [128, NSC*ET*512]: xtp[p, (c*ET+e)*512+s] = x[c*512+s, e*128+p].

    Chunk-major so each 512-wide s-chunk is one contiguous 8KB-per-partition
    DMA into the [P, NSC, ET, 512] SBUF tile.
    """
    x4 = x.reshape(S // 512, 512, E // P, P)          # [c, s, e, p]
    return np.ascontiguousarray(
        x4.transpose(3, 0, 2, 1).reshape(P, -1))      # [p, c, e, s]


def kernel(inputs, Wq, bq, Wk, bk, Wv, bv, _trace=False, _tmpdir=None):
    inputs = np.asarray(inputs, dtype=np.float32)
    wqp = _tile_by_h(np.asarray(Wq, dtype=np.float32).astype(NPBF16))
    wkp = _tile_by_h(np.asarray(Wk, dtype=np.float32).astype(NPBF16))
    wv_b = np.ascontiguousarray(np.asarray(Wv, dtype=np.float32).astype(NPBF16))
    bq_f = np.ascontiguousarray(bq, dtype=np.float32)
    bk_f = np.ascontiguousarray(bk, dtype=np.float32)
    bvb = np.ascontiguousarray(
        np.broadcast_to(np.asarray(bv, dtype=np.float32).astype(NPBF16),
                        (P, H)))
    nc = build_program()
    in_maps = []
    for c in range(N_CORES):
        in_maps.append({
            "xt": _pack_xt(inputs[c].astype(NPBF16)),
            "wqp": wqp, "bq": bq_f,
            "wkp": wkp, "bk": bk_f,
            "wv": wv_b, "bvb": bvb,
        })
    res = bass_utils.run_bass_kernel_spmd(
        nc, in_maps, core_ids=list(range(N_CORES)),
        trace=_trace, tmpdir=_tmpdir)
    out = np.stack([res.results[c]["out"].astype(np.float32)
                    for c in range(N_CORES)], axis=0)
    if _trace:
        kernel.last_results = res
    return out


# revision 42
# speedup vs baseline: 1.0157x; 1.0157x over previous
"""Single-head causal attention kernel for Trainium2 (Bass/Tile), SPMD over 8 cores.

Problem: inputs [B=8, S=2048, E=1024]; Wq/Wk/Wv [E, H=1024]; bq/bk/bv [H].
  q = x@Wq+bq; k = x@Wk+bk; v = x@Wv+bv
  out = softmax(causal(q k^T / sqrt(H))) v        -> [B, S, H]

Sharding: data-parallel over batch, 1 batch element per NeuronCore (8 cores).

Strategy (v3, bf16): host passes x pre-transposed (xT [E,S]) and weights in
bf16 (Wq/Wk additionally pre-tiled by output h-tile so the first matmul only
depends on a 256KB DMA), so the device does zero transposes and keeps
everything resident in SBUF:
  phase 1 (per 512-wide s-chunk): K^T[h,s] and Q^T[h,s] (stationary W tiles,
           bias fused into the PSUM eviction), then V[s,h] (stationary xT
           tiles; bias added during eviction from a host-broadcast [128,H]
           bv tile on GpSimd). All matmuls N=512.
  phase 2 (q-chunks processed in reverse so the tail chain is the smallest):
           scoresT[k,q] matmuls, exp(x/32) on ScalarE, causal edge mask via
           gpsimd.affine_select; AV + Z share one stationary load per attnT
           tile (fully-masked diagonal tiles skipped for even q-subtiles);
           1/Z folded into the PSUM eviction. AV of a chunk is issued after
           the next chunk's scores so the PE in-order queue never stalls on
           ScalarE.
  A short burst of dummy matmuls runs during the initial DMA wait to lift
  the PE HAM clock-gate to 8/8 before real work starts.
"""

import numpy as np
import ml_dtypes

import concourse.bacc as bacc
import concourse.mybir as mybir
from concourse import tile
from concourse import bass_utils

P = 128
F32 = mybir.dt.float32
BF16 = mybir.dt.bfloat16

B, S, E, H = 8, 2048, 1024, 1024
QC = 256          # q-chunk width in attention phase
N_CORES = 8
NPBF16 = ml_dtypes.bfloat16


def attention_kernel(tc, out, xt, wqp, bq, wkp, bk, wv, bvb):
    nc = tc.nc
    ST, ET, HT = S // P, E // P, H // P     # 128-tiles per dim
    NSC = S // 512                          # 512-wide s-chunks
    NQC = S // QC                           # q-chunks
    QSUB = QC // P
    inv_sqrt_h = 1.0 / float(np.sqrt(H))

    from contextlib import ExitStack

    root = ExitStack()
    with root:
        # ---- constants ----
        const = root.enter_context(tc.tile_pool(name="const", bufs=1))
        ones_col = const.tile([P, 1], BF16, name="ones_col")
        nc.gpsimd.memset(ones_col, 1.0)
        warm_src = const.tile([P, 512], BF16, name="warm_src")
        nc.gpsimd.memset(warm_src, 0.0)
        bk_sb = const.tile([P, HT], F32, name="bk_sb")
        nc.sync.dma_start(bk_sb[:], bk.rearrange("(t p) -> p t", p=P))
        bq_sb = const.tile([P, HT], F32, name="bq_sb")
        nc.sync.dma_start(bq_sb[:], bq.rearrange("(t p) -> p t", p=P))
        bv_sb = const.tile([P, H], BF16, name="bv_sb")

        # ---- resident arrays ----
        kqv_pool = root.enter_context(tc.tile_pool(name="kqv", bufs=1))
        kt = kqv_pool.tile([P, HT, S], BF16, name="kt")     # K^T [h,s]
        qt = kqv_pool.tile([P, HT, S], BF16, name="qt")     # Q^T [h,s]
        v_sb = kqv_pool.tile([P, ST, H], BF16, name="v_sb")  # V [s,h]

        # ================= phase 1: projections =================
        with ExitStack() as ph1:
            w_pool = ph1.enter_context(tc.tile_pool(name="w", bufs=1))
            # wk_sb[:, t, e, :] = Wk[e*128+p, t*128+c]  (host pre-tiled)
            wk_sb = w_pool.tile([P, HT, ET, P], BF16, name="wk_sb")
            wq_sb = w_pool.tile([P, HT, ET, P], BF16, name="wq_sb")
            wv_sb = w_pool.tile([P, ET, H], BF16, name="wv_sb")
            xt_pool = ph1.enter_context(tc.tile_pool(name="xt", bufs=1))
            # chunk-major: [:, c, e, :] is one contiguous 8KB/partition DMA
            xt_sb = xt_pool.tile([P, NSC, ET, 512], BF16, name="xt_sb")

            # DMA priority order (single queue => sequential arrival):
            # xt chunk0 + wk (first K matmuls), then wv, wq, xt chunks 1-3.
            CW = ET * 512                        # xtp columns per chunk
            # chunk 0 split per e-tile for fine-grained matmul wakeup
            nc.sync.dma_start(xt_sb[:, 0, 0, :], xt[:, 0:512])
            nc.sync.dma_start(wk_sb[:, 0, :, :], wkp[0:P, :])
            for e in range(1, ET):
                nc.sync.dma_start(xt_sb[:, 0, e, :],
                                  xt[:, e * 512:(e + 1) * 512])
            for t in range(1, HT):
                nc.sync.dma_start(wk_sb[:, t, :, :], wkp[t * P:(t + 1) * P, :])
            for e in range(ET):
                nc.sync.dma_start(wv_sb[:, e, :], wv[e * P:(e + 1) * P, :])
            nc.sync.dma_start(bv_sb[:], bvb)
            for t in range(HT):
                nc.sync.dma_start(wq_sb[:, t, :, :], wqp[t * P:(t + 1) * P, :])
            for c in range(1, NSC):
                nc.sync.dma_start(xt_sb[:, c, :, :],
                                  xt[:, c * CW:(c + 1) * CW])

            # kqpsum first: phase-2 pools then reuse banks that phase 1
            # frees early (after Q) rather than last (after V's evictions).
            kqpsum = ph1.enter_context(tc.tile_pool(name="kqpsum", bufs=2,
                                                    space="PSUM"))
            vpsum = ph1.enter_context(tc.tile_pool(name="vpsum", bufs=2,
                                                   space="PSUM"))
            # HAM warmup: dummy matmuls with no DMA dependency fill the
            # initial DMA wait and lift the PE clock gate to 8/8.
            wp = kqpsum.tile([P, 512], F32, name="kq0", space="PSUM")
            for _ in range(28):
                nc.tensor.matmul(wp[:], warm_src[:, 0:P], warm_src[:],
                                 start=True, stop=True)

            def evict_kq(dst, t, c, psum, bias, alt):
                if alt % 2 == 0:
                    nc.scalar.activation(
                        dst[:, t, c * 512:(c + 1) * 512], psum[:],
                        mybir.ActivationFunctionType.Identity,
                        bias=bias[:, t:t + 1])
                else:
                    nc.vector.tensor_scalar_add(
                        dst[:, t, c * 512:(c + 1) * 512], psum[:],
                        bias[:, t:t + 1])

            def v_chunk(c):
                for si in range(4 * c, 4 * c + 4):
                    vps = []
                    for hc in range(2):
                        vp = vpsum.tile([P, 512], F32, name="vp", space="PSUM")
                        for e in range(ET):
                            nc.tensor.matmul(
                                vp[:],
                                xt_sb[:, c, e, (si % 4) * P:(si % 4 + 1) * P],
                                wv_sb[:, e, hc * 512:(hc + 1) * 512],
                                start=(e == 0), stop=(e == ET - 1))
                        vps.append(vp)
                    for hc in range(2):
                        nc.vector.scalar_tensor_tensor(
                            v_sb[:, si, hc * 512:(hc + 1) * 512], vps[hc][:],
                            1.0, bv_sb[:, hc * 512:(hc + 1) * 512],
                            mybir.AluOpType.mult, mybir.AluOpType.add)

            # ---- chunk 0: chunk-local (DMA-paced at startup) ----
            for t in range(HT):
                kp = kqpsum.tile([P, 512], F32, name="kq0", space="PSUM")
                for e in range(ET):
                    nc.tensor.matmul(
                        kp[:], wk_sb[:, t, e, :], xt_sb[:, 0, e, :],
                        start=(e == 0), stop=(e == ET - 1))
                evict_kq(kt, t, 0, kp, bk_sb, t)
            v_chunk(0)
            for t in range(HT):
                qp = kqpsum.tile([P, 512], F32, name="kq1", space="PSUM")
                for e in range(ET):
                    nc.tensor.matmul(
                        qp[:], wq_sb[:, t, e, :], xt_sb[:, 0, e, :],
                        start=(e == 0), stop=(e == ET - 1))
                evict_kq(qt, t, 0, qp, bq_sb, t)
            # ---- chunks 1-3: reuse each stationary W tile across chunks ----
            for dst, w_sb, b_sb in ((kt, wk_sb, bk_sb), (qt, wq_sb, bq_sb)):
                for t in range(HT):
                    kps = [kqpsum.tile([P, 512], F32, name=f"kq{cc}",
                                       space="PSUM") for cc in range(3)]
                    for e in range(ET):
                        for cc in range(3):
                            nc.tensor.matmul(
                                kps[cc][:], w_sb[:, t, e, :],
                                xt_sb[:, 1 + cc, e, :],
                                start=(e == 0), stop=(e == ET - 1))
                    for cc in range(3):
                        evict_kq(dst, t, 1 + cc, kps[cc], b_sb, t + cc)
            for c in range(1, NSC):
                v_chunk(c)

        # ================= phase 2: attention =================
        with ExitStack() as ph2:
            attn_pool = ph2.enter_context(
                tc.tile_pool(name="attnT", bufs=36))
            o_pool = ph2.enter_context(tc.tile_pool(name="o_stage", bufs=3))
            rz_pool = ph2.enter_context(tc.tile_pool(name="rz", bufs=4))
            spsum = ph2.enter_context(tc.tile_pool(name="spsum", bufs=2,
                                                   space="PSUM"))
            zpsum = ph2.enter_context(tc.tile_pool(name="zpsum", bufs=2,
                                                   space="PSUM"))
            opsum = ph2.enter_context(tc.tile_pool(name="opsum", bufs=4,
                                                   space="PSUM"))

            def scores_chunk(j):
                """ScoresT tiles [k,q] + exp + causal mask for q-chunk j."""
                nk = ((j + 1) * QC) // P
                ats = []
                for i in range(nk):
                    # The last k-tile (i == 2j+1) lies above the diagonal for
                    # the first q-subtile; only its [:, P:2P] half is ever
                    # read by AV/Z, so compute just those 128 columns.
                    lo = P if i == nk - 1 else 0
                    sp = spsum.tile([P, QC], F32, name="sp", space="PSUM")
                    for t in range(HT):
                        nc.tensor.matmul(
                            sp[:, 0:QC - lo],
                            kt[:, t, i * P:(i + 1) * P],
                            qt[:, t, j * QC + lo:(j + 1) * QC],
                            start=(t == 0), stop=(t == HT - 1))
                    at = attn_pool.tile([P, QC], BF16, name="at")
                    nc.scalar.activation(at[:, lo:QC], sp[:, 0:QC - lo],
                                         mybir.ActivationFunctionType.Exp,
                                         scale=inv_sqrt_h)
                    if (i + 1) * P > j * QC + lo:   # tile touches the diagonal
                        nc.gpsimd.affine_select(
                            out=at[:, lo:QC], in_=at[:, lo:QC],
                            compare_op=mybir.AluOpType.is_ge,
                            fill=0.0,
                            base=j * QC + lo - i * P,
                            channel_multiplier=-1,
                            pattern=[[1, QC - lo]])
                    ats.append(at)
                return ats

            def av_chunk(j, ats):
                """AV + Z for q-chunk j given its masked attnT tiles."""
                for qs in range(QSUB):
                    # causal: k-tiles above the diagonal for this q-subtile
                    # are fully masked; skip them.
                    nk = 2 * j + qs + 1
                    zp = zpsum.tile([P, 1], F32, name="zp", space="PSUM")
                    ops = [opsum.tile([P, 512], F32, name="op", space="PSUM")
                           for _ in range(2)]
                    for i in range(nk):
                        a_sl = ats[i][:, qs * P:(qs + 1) * P]
                        for hc in range(2):
                            nc.tensor.matmul(
                                ops[hc][:], a_sl,
                                v_sb[:, i, hc * 512:(hc + 1) * 512],
                                start=(i == 0), stop=(i == nk - 1))
                        nc.tensor.matmul(zp[:], a_sl, ones_col[:, :],
                                         start=(i == 0), stop=(i == nk - 1))
                    rz = rz_pool.tile([P, 1], F32, name="rz")
                    nc.vector.reciprocal(rz[:], zp[:])
                    o_st = o_pool.tile([P, H], BF16, name="o_st")
                    row = j * QC + qs * P
                    for hc in range(2):
                        nc.vector.tensor_scalar_mul(
                            o_st[:, hc * 512:(hc + 1) * 512], ops[hc][:],
                            rz[:, 0:1])
                        nc.sync.dma_start(
                            out[row:row + P, hc * 512:(hc + 1) * 512],
                            o_st[:, hc * 512:(hc + 1) * 512])

            prev = None
            prev_j = None
            for j in range(NQC - 1, -1, -1):     # reverse: smallest AV last
                ats = scores_chunk(j)
                if prev is not None:
                    av_chunk(prev_j, prev)
                prev, prev_j = ats, j
            av_chunk(prev_j, prev)


def build_program(n_cores=N_CORES):
    nc = bacc.Bacc("TRN2", target_bir_lowering=False, debug=False,
                   num_devices=n_cores)
    xt = nc.dram_tensor("xt", [P, S * E // P], BF16, kind="ExternalInput").ap()
    wqp = nc.dram_tensor("wqp", [H, E], BF16, kind="ExternalInput").ap()
    bq = nc.dram_tensor("bq", [H], F32, kind="ExternalInput").ap()
    wkp = nc.dram_tensor("wkp", [H, E], BF16, kind="ExternalInput").ap()
    bk = nc.dram_tensor("bk", [H], F32, kind="ExternalInput").ap()
    wv = nc.dram_tensor("wv", [E, H], BF16, kind="ExternalInput").ap()
    bvb = nc.dram_tensor("bvb", [P, H], BF16, kind="ExternalInput").ap()
    out = nc.dram_tensor("out", [S, H], BF16, kind="ExternalOutput").ap()
    with tile.TileContext(nc) as tc:
        attention_kernel(tc, out, xt, wqp, bq, wkp, bk, wv, bvb)
    nc.compile()
    return nc


def _tile_by_h(w):
    """[E,H] -> [H,E] layout where row t*128+p, col e*128+c = w[e*128+p, t*128+c].

    So a [128, E] slice at row offset t*128 holds, for partition p, the
    concatenation over e of Wk[e*128+p, t*128:(t+1)*128].
    """
    w4 = w.reshape(E // P, P, H // P, P)          # [e, p, t, c]
    return np.ascontiguousarray(
        w4.transpose(2, 1, 0, 3).reshape(H, E))   # [t, p, e, c] -> [H, E]


def _pack_xt(x):
    """x [S,E] -> xtp [128, NSC*ET*512]: xtp[p, (c*ET+e)*512+s] = x[c*512+s, e*128+p].

    Chunk-major so each 512-wide s-chunk is one contiguous 8KB-per-partition
    DMA into the [P, NSC, ET, 512] SBUF tile.
    """
    x4 = x.reshape(S // 512, 512, E // P, P)          # [c, s, e, p]
    return np.ascontiguousarray(
        x4.transpose(3, 0, 2, 1).reshape(P, -1))      # [p, c, e, s]


def kernel(inputs, Wq, bq, Wk, bk, Wv, bv, _trace=False, _tmpdir=None):
    inputs = np.asarray(inputs, dtype=np.float32)
    wqp = _tile_by_h(np.asarray(Wq, dtype=np.float32).astype(NPBF16))
    wkp = _tile_by_h(np.asarray(Wk, dtype=np.float32).astype(NPBF16))
    wv_b = np.ascontiguousarray(np.asarray(Wv, dtype=np.float32).astype(NPBF16))
    bq_f = np.ascontiguousarray(bq, dtype=np.float32)
    bk_f = np.ascontiguousarray(bk, dtype=np.float32)
    bvb = np.ascontiguousarray(
        np.broadcast_to(np.asarray(bv, dtype=np.float32).astype(NPBF16),
                        (P, H)))
    nc = build_program()
    in_maps = []
    for c in range(N_CORES):
        in_maps.append({
            "xt": _pack_xt(inputs[c].astype(NPBF16)),
            "wqp": wqp, "bq": bq_f,
            "wkp": wkp, "bk": bk_f,
            "wv": wv_b, "bvb": bvb,
        })
    res = bass_utils.run_bass_kernel_spmd(
        nc, in_maps, core_ids=list(range(N_CORES)),
        trace=_trace, tmpdir=_tmpdir)
    out = np.stack([res.results[c]["out"].astype(np.float32)
                    for c in range(N_CORES)], axis=0)
    if _trace:
        kernel.last_results = res
    return out
